# revision 7
# baseline (speedup 1.0000x reference)
"""CTC loss (keras ctc_batch_cost semantics, full lengths) on 8 Trainium2 cores.

Strategy (data parallel, B=512 -> 64 samples/core):
- Exp-space DP with periodic max-rescaling; partitions 0-63 run the forward
  DP (t=0..255), partitions 64-127 the backward DP (t=511..256) in reversed
  state order (identical recurrence) -> 256 unified steps + small combine.
- State reformulation: Y[c] = E[c] + O[c-1] (blank-lattice partial sums) and
  Ox[c] = O[c-1], interleaved as (Ox[c], Y[c]) pairs in one [128, 258] tile.
  One step:
      W[c]  = Y[c] - mbar[c]*Ox[c]        (skip-mask correction)
      t[c]  = W[c-1] + Ox[c]
      Ox'[c] = |ghat[c] * t[c]|           (ghat = +-p_label, sign = mbar)
      Y'[c]  = pb*Y[c] + Ox'[c]           (pb = per-sample blank prob scalar)
- The whole step is ONE custom DVE instruction (hand-written 2-phase uop
  program, registered per-NEFF): in0 = interleaved state (258 elems), in1 =
  ghat (129 elems, phase-A-only pops), C0 = pb. Cross-element handoffs ride
  the per-stage CURR_ALU_OUT flops; the skip mask rides the sign bit of the
  gathered label probs (negated copy of the raw tile, gather index +400).
- Gather: per-sample label indices baked into uint16 tables (host prep),
  GPSIMD indirect_copy in the baseline "octet" layout, SBUF->SBUF DMA repack
  into [zero, +-labels(128), pb] chunks of 130 per timestep.
"""

import numpy as np

import concourse.bass as bass
import concourse.bacc as bacc
import concourse.tile as tile
from concourse import mybir
from concourse._compat import get_trn_type
from concourse.bass_utils import run_bass_kernel_spmd

F32 = mybir.dt.float32
U16 = mybir.dt.uint16
ALU = mybir.AluOpType
AF = mybir.ActivationFunctionType
AX = mybir.AxisListType

B, T, C, L = 512, 512, 100, 128
BLANK = C - 1
EPS = 1e-7
NCORES = 8
BPC = B // NCORES          # 64 samples per core
NW = 4                     # windows over the 256 unified steps
WSLOTS = 256 // NW         # 64 slots per window
WP = WSLOTS // 16          # 4 t-rows per partition per octet call
NCALL = 16                 # octet calls per window (8 fwd + 8 bwd)
SW = L + 2                 # 130: gwin slot chunk [zero, +-labels(128), pb]
NIDX = WP * SW             # 520 gather indices per group per call
IDXC = ((NIDX + 15) // 16 + 1) // 2 * 2  # idx cols per call (even)
RAWC = WP * C              # 400 raw cols; negated copy at +400; zeros at 800
RESC = 64                  # rescale cadence
RREV = 132                 # reversal gather entries (mult of 4; 129 used)
RIDXC = ((RREV + 15) // 16 + 1) // 2 * 2   # reversal idx cols (even)

# ------------------------------------------------------ custom DVE step op
_CTC_OP = None


def _ctc_step_ref(in0, in1, c0, c1, c2):
    """Numpy reference for CoreSim: one CTC step over interleaved state."""
    in0 = np.asarray(in0, np.float64)
    P = in0.shape[0]
    st = in0.reshape(P, -1)
    S = st.shape[1] // 2
    Ox0, Y0 = st[:, 0::2], st[:, 1::2]
    g = np.asarray(in1, np.float64).reshape(P, -1)[:, : S]
    pb = np.asarray(c0, np.float64).reshape(P, 1)
    mb = (g < 0).astype(np.float64)
    W0 = Y0 - mb * Ox0
    t = np.concatenate([np.zeros((P, 1)), W0[:, :-1]], axis=1) + Ox0
    Ox1 = np.abs(g * t)
    Y1 = pb * Y0 + Ox1
    out = np.empty_like(st)
    out[:, 0::2] = Ox1
    out[:, 1::2] = Y1
    return out


def _build_ctc_uops():
    from concourse.dve_uop import (
        ENABLE, DISABLE, AluInp, AluOp, DelayInp, InpSel, OutPath, OutSel,
        Trigger, UopConfig, UopDpConfig,
    )

    def phase_a():
        u = UopConfig()
        u.enable_input(InpSel.SRC_0, 0)      # Ox0[c]
        u.enable_input(InpSel.SRC_1, 1)      # ghat[c] -> delay_0
        dp = [UopDpConfig() for _ in range(8)]
        dp[0].enable_alu(AluOp.BYPASS, AluInp.PREV_ALU_OUT).pass_through_delay(0)
        dp[1].enable_alu(AluOp.BYPASS, AluInp.PREV_DELAY_0)
        dp[1].enable_delay_from_src(DelayInp.PREV_ALU_OUT, 1)
        dp[1].pass_through_delay(0)
        dp[2].enable_alu(AluOp.BYPASS, AluInp.PREV_ALU_OUT).pass_through_delay(0, 1)
        dp[3].enable_alu(AluOp.ADD, AluInp.PREV_DELAY_1, AluInp.CURR_ALU_OUT)
        dp[3].pass_through_delay(0)
        dp[4].enable_alu(AluOp.MULTIPLY, AluInp.PREV_ALU_OUT, AluInp.PREV_DELAY_0)
        dp[5].enable_alu(AluOp.ABSOLUTE_VALUE, AluInp.PREV_ALU_OUT)
        dp[6].enable_alu(AluOp.BYPASS, AluInp.PREV_ALU_OUT)
        dp[7].enable_alu(AluOp.BYPASS, AluInp.PREV_ALU_OUT)
        u.datapath_config = dp
        u.require_inp0 = ENABLE
        u.require_inp1 = ENABLE
        u.enable_output(OutSel.ALU_OUT, OutPath.WR0_LO)
        u.repeat_count = 1
        return u

    def phase_b():
        u = UopConfig()
        u.enable_input(InpSel.SRC_0, 0)      # Y0[c]
        u.enable_input(InpSel.SRC_0, 2)      # Y0[c] -> delay_1
        u.enable_input(InpSel.CONST_0, 3)    # pb -> delay_2
        u.enable_input(InpSel.ZERO, 4)       # 0.0 -> delay_3
        dp = [UopDpConfig() for _ in range(8)]
        dp[0].enable_alu(AluOp.BYPASS, AluInp.CURR_ALU_OUT)
        dp[0].pass_through_delay(1, 2, 3)
        dp[1].enable_alu(AluOp.IS_LT, AluInp.CURR_ALU_OUT, AluInp.PREV_DELAY_3)
        dp[1].enable_delay_from_src(DelayInp.PREV_ALU_OUT, 4)
        dp[1].pass_through_delay(1, 2)
        dp[2].enable_alu(AluOp.MULTIPLY, AluInp.PREV_ALU_OUT, AluInp.PREV_DELAY_4)
        dp[2].pass_through_delay(1, 2)
        dp[3].enable_alu(AluOp.SUBTRACT, AluInp.PREV_DELAY_1, AluInp.PREV_ALU_OUT)
        dp[3].pass_through_delay(1, 2)
        dp[4].enable_alu(AluOp.MULTIPLY, AluInp.PREV_DELAY_1, AluInp.PREV_DELAY_2)
        dp[5].enable_alu(AluOp.ADD, AluInp.PREV_ALU_OUT, AluInp.CURR_ALU_OUT)
        dp[6].enable_alu(AluOp.BYPASS, AluInp.PREV_ALU_OUT)
        dp[7].enable_alu(AluOp.BYPASS, AluInp.PREV_ALU_OUT)
        u.datapath_config = dp
        u.require_inp0 = ENABLE
        u.require_inp1 = DISABLE
        u.enable_output(OutSel.ALU_OUT, OutPath.WR0_LO)
        u.repeat_count = 1
        return u

    a0 = phase_a()
    a0.trigger = (Trigger.COUNT, Trigger.NONE, Trigger.NONE)
    a0.next_uop = (1, 0, 0)
    b = phase_b()
    b.trigger = (Trigger.SRC_TENSOR_DONE, Trigger.COUNT, Trigger.NONE)
    b.next_uop = (0, 2, 0)
    a = phase_a()
    a.trigger = (Trigger.SRC_TENSOR_DONE, Trigger.COUNT, Trigger.NONE)
    a.next_uop = (0, 1, 0)
    return [a0, b, a]


def _get_ctc_op():
    """Register the hand-written step op with dve_ops (idempotent)."""
    global _CTC_OP
    if _CTC_OP is not None:
        return _CTC_OP
    import concourse.dve_ops as dve_ops
    from concourse.dve_spec import Spec, Src0, Src1
    from concourse.dve_uop import DveOpSpec

    name = "CTC_STEP_ANT"
    if name not in dve_ops._SUB_OPCODE_FOR_NAME:
        row = dve_ops._CUSTOM_DVE_ROW_BASE + len(dve_ops.OPS)
        assert row < 0x20
        spec = Spec(body=Src0 + Src1, reference=_ctc_step_ref)
        op = dve_ops.DveOp(name=name, spec=spec, subdim=False, uops_sha={})
        dve_ops.OPS.append(op)
        dve_ops._SUB_OPCODE_FOR_NAME[name] = row
        dve_ops.CUSTOM_DVE_SPECS[name] = spec
        for ver in ("v3", "v4"):
            ds = DveOpSpec(
                name=name, opcode=row, uops=_build_ctc_uops(), rd1_en=True
            )
            ds.validate(ver)
            dve_ops._COMPILE_CACHE[(name, ver)] = ds
    _CTC_OP = next(o for o in dve_ops.OPS if o.name == name)
    return _CTC_OP


# ----------------------------------------------------------------- host prep
def _host_tables(y_true_core):
    """Index/mask tables from labels. y_true_core: (64, L) int."""
    lab = y_true_core.astype(np.int64)
    lrev = lab[:, ::-1]
    mF = np.zeros((BPC, L), np.float32)
    mF[:, 1:] = (lab[:, 1:] != lab[:, :-1]).astype(np.float32)
    mcomb = np.zeros((128, L), np.float32)
    mcomb[0:64, : L - 1] = mF[:, 1:]                     # combine: mF_ext[j+1]

    # gather index tables: 16 calls x (128, IDXC) packed as (128, 16*IDXC).
    # Entry for label i: raw col q*C+lab[i], +400 (negated copy) when
    # lab[i+1]==lab[i] (the skip into lattice column i+1 is forbidden).
    gidx = np.zeros((128, NCALL * IDXC), np.uint16)
    for o in range(NCALL):
        fwd = o < 8
        for g in range(8):
            s = 8 * o + g if fwd else 8 * (o - 8) + g
            labs = lab[s] if fwd else lrev[s]
            mbar = np.zeros(L, np.int64)
            mbar[: L - 1] = (labs[1:] == labs[:-1]).astype(np.int64)
            stream = np.empty(NIDX, np.uint16)
            for wl in range(WP):
                q = wl if fwd else (WP - 1 - wl)
                stream[wl * SW] = 2 * RAWC          # zero column
                stream[wl * SW + 1: wl * SW + 1 + L] = \
                    q * C + labs + 400 * mbar
                stream[wl * SW + 1 + L] = q * C + BLANK
            for i in range(NIDX):
                gidx[16 * g + i % 16, o * IDXC + i // 16] = stream[i]

    # reversal indices (same stream for every 16-partition group): j -> 128-j,
    # padded to RREV=132 entries (multiple of 4 for the gpsimd gather ucode)
    ridx = np.zeros((128, RIDXC), np.uint16)
    for g in range(8):
        for i in range(RREV):
            ridx[16 * g + i % 16, i // 16] = max(L - i, 0)
    return gidx, ridx, mcomb


# ------------------------------------------------------------- bass program
_PROGRAM = None


def _build_program(snap_ks=(), nsteps=256, null=False, reps=1):
    if null:
        nc = bacc.Bacc(get_trn_type() or "TRN2", target_bir_lowering=False,
                       debug=False, enable_asserts=False)
        loss_d = nc.dram_tensor("loss", [BPC, 1], F32, kind="ExternalOutput").ap()
        with tile.TileContext(nc) as tc:
            with tc.tile_pool(name="p", bufs=1) as pool:
                t = pool.tile([BPC, 1], F32, name="nullt")
                nc.vector.memset(t[:], 0.0)
                nc.sync.dma_start(loss_d[:], t[:])
        nc.compile()
        return nc
    ctc_op = _get_ctc_op()
    nc = bacc.Bacc(get_trn_type() or "TRN2", target_bir_lowering=False,
                   debug=False, enable_asserts=False)
    snaps = {}
    for k in snap_ks:
        snaps[f"snapS_{k}"] = nc.dram_tensor(
            f"snapS_{k}", [128, 2 * (L + 1)], F32, kind="ExternalOutput").ap()
        snaps[f"snapacc_{k}"] = nc.dram_tensor(
            f"snapacc_{k}", [128, 1], F32, kind="ExternalOutput").ap()
    if snap_ks:
        snaps["snapgw_0"] = nc.dram_tensor(
            "snapgw_0", [128, WSLOTS * SW], F32, kind="ExternalOutput").ap()

    yp = nc.dram_tensor("yp", [BPC, T, C], F32, kind="ExternalInput").ap()
    gidx_d = nc.dram_tensor("gidx", [128, NCALL * IDXC], U16,
                            kind="ExternalInput").ap()
    ridx_d = nc.dram_tensor("ridx", [128, RIDXC], U16,
                            kind="ExternalInput").ap()
    mcomb_d = nc.dram_tensor("mcomb", [128, L], F32,
                             kind="ExternalInput").ap()
    loss_d = nc.dram_tensor("loss", [BPC, 1], F32, kind="ExternalOutput").ap()

    with tile.TileContext(nc) as tc:
        with (
            tc.tile_pool(name="consts", bufs=1) as consts,
            tc.tile_pool(name="raw", bufs=4) as rawp,
            tc.tile_pool(name="gout", bufs=4) as goutp,
            tc.tile_pool(name="gwin", bufs=2) as gwinp,
            tc.tile_pool(name="pbe", bufs=2) as pbep,
            tc.tile_pool(name="state", bufs=1) as statep,
            tc.tile_pool(name="small", bufs=2) as smallp,
        ):
            # constants
            gidx_s = consts.tile([128, NCALL * IDXC], U16, tag="gidx")
            ridx_s = consts.tile([128, RIDXC], U16, tag="ridx")
            mcb = consts.tile([128, L], F32, tag="mcb")
            nc.sync.dma_start(gidx_s[:], gidx_d[:])
            nc.sync.dma_start(ridx_s[:], ridx_d[:])
            nc.sync.dma_start(mcb[:], mcomb_d[:])

            # persistent interleaved state (Ox[c], Y[c]) pairs, ping-pong
            Ss = [statep.tile([128, 2 * (L + 1)], F32, name=f"S{i}", tag=f"S{i}")
                  for i in range(2)]
            acc = statep.tile([128, 1], F32, tag="acc")
            dumS = statep.tile([128, 4], F32, tag="dumS")
            dumG = statep.tile([128, 2], F32, tag="dumG")

            # pre-zero the spare column of the 4 rotating raw buffers (the
            # gather's ghat[0]=0 source); the loop never writes cols 800:802.
            raw_bufs = [rawp.tile([128, 2 * RAWC + 2], F32,
                                  name=f"rawpre{i}", tag="raw")
                        for i in range(4)]
            for rb in raw_bufs:
                nc.vector.memset(rb[:, 2 * RAWC:], 0.0)

          # ---- per-iteration body (reps>1 used only for timing) ----
            for _rep in range(reps):
                for t_ in (Ss[0], Ss[1], dumS, dumG):
                    nc.vector.memset(t_[:], 0.0)
                nc.vector.memset(acc[:], 0.0)
                nc.vector.memset(Ss[0][:, 1:2], 1.0)   # Y[0] = E[0] = 1
                # flush NaN garbage out of the per-stage CURR flops
                nc.vector._custom_dve(ctc_op, out=dumS[:], in0=dumS[:],
                                      in1=dumG[:], s0=0.0)

                # window prep: load + negate + gather + repack
                gwins, pbes = [], []
                for w in range(NW):
                    gwin = gwinp.tile([128, WSLOTS * SW], F32, tag="gwin")
                    for o in range(NCALL):
                        raw = rawp.tile([128, 2 * RAWC + 2], F32, tag="raw")
                        if o < 8:
                            s0 = 8 * o
                            src = (yp[s0:s0 + 8, w * WSLOTS: (w + 1) * WSLOTS, :]
                                   .rearrange("s (r q) c -> s r (q c)", r=16))
                        else:
                            s0 = 8 * (o - 8)
                            t_lo = 512 - (w + 1) * WSLOTS
                            src = (yp[s0:s0 + 8, t_lo: t_lo + WSLOTS, :]
                                   .rearrange("s (r q) c -> s r (q c)", r=16)
                                   [:, ::-1, :])
                        nc.sync.dma_start(raw[:, 0:RAWC], src)
                        nc.scalar.mul(raw[:, RAWC:2 * RAWC], raw[:, 0:RAWC],
                                      -1.0)
                        gout = goutp.tile([128, NIDX], F32, tag="gout")
                        nc.gpsimd.indirect_copy(
                            gout[:], raw[:],
                            gidx_s[:, o * IDXC:(o + 1) * IDXC], True)
                        row0 = 8 * o if o < 8 else 64 + 8 * (o - 8)
                        nc.sync.dma_start(
                            gwin[row0:row0 + 8, :], gout[:])
                    pbe = pbep.tile([128, WSLOTS], F32, tag="pbe")
                    gw3 = gwin[:].rearrange("p (s c) -> p s c", c=SW)
                    nc.vector.tensor_scalar_add(
                        pbe[:], gw3[:, :, SW - 1:SW].squeeze(2), float(EPS))
                    gwins.append(gwin)
                    pbes.append(pbe)
                    if snap_ks and w == 0:
                        nc.sync.dma_start(snaps["snapgw_0"][:], gwin[:])

                # unified DP: 256 steps, one custom-DVE instruction each
                cur = 0
                for k in range(nsteps):
                    w, slot = divmod(k, WSLOTS)
                    gwin, pbe = gwins[w], pbes[w]
                    nc.vector._custom_dve(
                        ctc_op,
                        out=Ss[1 - cur][:],
                        in0=Ss[cur][:],
                        in1=gwin[:, slot * SW: slot * SW + L + 1],
                        s0=pbe[:, slot:slot + 1],
                    )
                    cur = 1 - cur

                    if (k + 1) % RESC == 0:
                        Sc = Ss[cur]
                        rm = smallp.tile([128, 1], F32, tag="rm")
                        ri = smallp.tile([128, 1], F32, tag="ri")
                        lg = smallp.tile([128, 1], F32, tag="lg")
                        nc.vector.tensor_reduce(rm[:], Sc[:], axis=AX.X,
                                                op=ALU.max)
                        nc.vector.reciprocal(ri[:], rm[:])
                        nc.vector.tensor_scalar_mul(Sc[:], Sc[:], ri[:])
                        nc.scalar.activation(lg[:], ri[:], AF.Ln)
                        nc.vector.tensor_sub(acc[:], acc[:], lg[:])

                    if k in snap_ks:
                        nc.sync.dma_start(snaps[f"snapS_{k}"][:], Ss[cur][:])
                        nc.sync.dma_start(snaps[f"snapacc_{k}"][:], acc[:])

            # combine: recover Ef/Oxf from the interleaved state, then the
            # meet-in-the-middle dot product (identical math to the log-space
            # split: loss = -(ln(dot) + accF + accB)).
            Sf3 = Ss[cur][:].rearrange("p (s c) -> p s c", c=2)
            Oxf = statep.tile([128, L + 1], F32, tag="Oxf")
            Ef = statep.tile([128, L + 1], F32, tag="Ef")
            nc.vector.tensor_copy(Oxf[:], Sf3[:, :, 0:1].squeeze(2))
            nc.vector.tensor_sub(Ef[:], Sf3[:, :, 1:2].squeeze(2), Oxf[:])

            WEs = statep.tile([128, RREV], F32, tag="WEs")
            WOxs = statep.tile([128, RREV], F32, tag="WOxs")
            accB = statep.tile([64, 1], F32, tag="accB")
            RWE = statep.tile([128, RREV], F32, tag="RWE")
            RWOx = statep.tile([128, RREV], F32, tag="RWOx")
            nc.vector.memset(WEs[:], 0.0)
            nc.vector.memset(WOxs[:], 0.0)
            nc.sync.dma_start(WEs[0:64, 0:L + 1], Ef[64:128, :])
            nc.sync.dma_start(WOxs[0:64, 0:L + 1], Oxf[64:128, :])
            nc.sync.dma_start(accB[:], acc[64:128, :])
            nc.gpsimd.indirect_copy(RWE[:], WEs[:], ridx_s[:], True)
            nc.gpsimd.indirect_copy(RWOx[:], WOxs[:], ridx_s[:], True)

            betaE = statep.tile([64, L + 1], F32, tag="betaE")
            tb1 = statep.tile([64, L], F32, tag="tb1")
            tb2 = statep.tile([64, L], F32, tag="tb2")
            betaO = statep.tile([64, L], F32, tag="betaO")
            junkE = statep.tile([64, L + 1], F32, tag="junkE")
            junkO = statep.tile([64, L], F32, tag="junkO")
            dE = statep.tile([64, 1], F32, tag="dE")
            dO = statep.tile([64, 1], F32, tag="dO")
            ds = statep.tile([64, 1], F32, tag="ds")
            lg2 = statep.tile([64, 1], F32, tag="lg2")
            lnS = statep.tile([64, 1], F32, tag="lnS")
            tot = statep.tile([64, 1], F32, tag="tot")
            tot2 = statep.tile([64, 1], F32, tag="tot2")
            res = statep.tile([64, 1], F32, tag="res")

            nc.vector.tensor_add(betaE[:], RWE[0:64, 0:L + 1], RWOx[0:64, 0:L + 1])
            nc.vector.tensor_mul(tb1[:], mcb[0:64, :], RWOx[0:64, 1:L + 1])
            nc.vector.tensor_add(tb2[:], RWE[0:64, 1:L + 1], tb1[:])
            nc.vector.tensor_add(betaO[:], RWOx[0:64, 0:L], tb2[:])
            nc.vector.scalar_tensor_tensor(
                out=junkE[:], in0=Ef[0:64, :], scalar=1.0, in1=betaE[:],
                op0=ALU.mult, op1=ALU.mult, accum_out=dE[:])
            nc.vector.scalar_tensor_tensor(
                out=junkO[:], in0=Oxf[0:64, 1:], scalar=1.0, in1=betaO[:],
                op0=ALU.mult, op1=ALU.mult, accum_out=dO[:])
            nc.vector.tensor_add(ds[:], dE[:], dO[:])
            # ds can be far below 2^-64 (outside the ACT Ln LUT range), so
            # ln(ds) = 2*ln(sqrt(ds*2^20)) - 20*ln2 keeps the LUT in range.
            nc.scalar.activation(lg2[:], ds[:], AF.Sqrt, scale=float(2.0 ** 20))
            nc.scalar.activation(lnS[:], lg2[:], AF.Ln)
            nc.vector.tensor_add(tot[:], acc[0:64, :], accB[:])
            nc.vector.tensor_scalar_add(tot2[:], tot[:], float(-20.0 * np.log(2.0)))
            nc.vector.scalar_tensor_tensor(
                out=res[:], in0=lnS[:], scalar=-2.0, in1=tot2[:],
                op0=ALU.mult, op1=ALU.subtract)
            nc.sync.dma_start(loss_d[:], res[:])

    nc.compile()
    return nc


def _get_program():
    global _PROGRAM
    if _PROGRAM is None:
        _PROGRAM = _build_program()
    return _PROGRAM


def make_in_maps(y_true, y_pred):
    y_true = np.asarray(y_true)
    y_pred = np.ascontiguousarray(np.asarray(y_pred, dtype=np.float32))
    in_maps = []
    for c in range(NCORES):
        sl = slice(c * BPC, (c + 1) * BPC)
        gidx, ridx, mcomb = _host_tables(y_true[sl])
        in_maps.append({
            "yp": y_pred[sl],
            "gidx": gidx,
            "ridx": ridx,
            "mcomb": mcomb,
        })
    return in_maps


def kernel(y_true, y_pred):
    nc = _get_program()
    in_maps = make_in_maps(y_true, y_pred)
    res = run_bass_kernel_spmd(nc, in_maps, core_ids=list(range(NCORES)))
    out = np.concatenate([res.results[c]["loss"] for c in range(NCORES)], axis=0)
    return out.astype(np.float32)


if __name__ == "__main__":
    y_true = np.load("y_true.npy")
    y_pred = np.load("y_pred.npy")
    out = kernel(y_true, y_pred)
    exp = np.load("expected_np.npy")
    err = np.abs(out.ravel() - exp) / np.maximum(1.0, np.abs(exp))
    print("kernel out[:4]:", out.ravel()[:4])
    print("expected [:4]:", exp[:4])
    print("max rel err:", err.max())
